# revision 6
# baseline (speedup 1.0000x reference)
"""Bilateral anti-alias filter on Trainium2, 8-core data parallel.

Full inputs: images [16,3,512,512] f32, spatial_kernel [5,5] f32.
Shards the batch over 8 NeuronCores (2 images each), runs a Bass/Tile
kernel per core, gathers the full output.

Math (per pixel, K=5, sigma_i=0.1), using pair symmetry over the 12
offsets t=(di,dj) with di>=0 lexicographically positive:

  d_t = p(x+t) - p(x)
  e_t = exp(-50 d_t^2)  computed as Derivative_Erf(sqrt(50) d)*sqrt(pi)/2
  u_t = e_t * d_t
  den(x) = s0 + sum_t [s+_t e_t + s-_t shift_t(e_t)]
  pa(x)  = sum_t [s+_t u_t - s-_t shift_t(u_t)]
  out = p + pa / den

shift_t realized on the TensorEngine via banded lhsT matmuls into PSUM
(spatial weights folded into the lhsT values).  PSUM is drained fast by
ScalarE Identity/Copy (table-swap-free), the reciprocal runs on the
otherwise-idle GpSimd engine (bit-trick seed + 1 Newton step), and the
final multiply/add run on DVE.  Output is stored bf16 and upcast on the
host.
"""
import sys

sys.path.insert(0, "/opt/trn_rl_repo")

import math
import numpy as np
import ml_dtypes
from contextlib import ExitStack

import concourse.bass as bass
import concourse.tile as tile
from concourse import bacc, mybir
from concourse.bass_utils import run_bass_kernel_spmd

f32 = mybir.dt.float32
bf16 = mybir.dt.bfloat16
i32 = mybir.dt.int32
AF = mybir.ActivationFunctionType
Alu = mybir.AluOpType

N_CORES = 8
B_FULL, C, H, W = 16, 3, 512, 512
B_SH = B_FULL // N_CORES  # 2 images per core
KK = 5
PAD = KK // 2  # 2
SQ50 = float(np.sqrt(np.float32(50.0)))
C_DERF = 2.0 / math.sqrt(math.pi)  # Derivative_Erf(x) = C_DERF*exp(-x^2)
NOUT = 124  # output rows per band
NG = 128    # plane partitions (= NOUT + 4)
WB = W + 4  # 516: padded col buffer, tile col c <-> image col c-2
WIN = W + 2  # 514: per-pair plane window width

# 12 pairs (di, dj) with di >= 0, lexicographically positive
PAIRS = [
    (0, 1), (0, 2),
    (1, -2), (1, -1), (1, 0), (1, 1), (1, 2),
    (2, -2), (2, -1), (2, 0), (2, 1), (2, 2),
]
BATCHES = [PAIRS[0:4], PAIRS[4:8], PAIRS[8:12]]
NB = 4  # pairs per batch


def _jbase(dj):
    """Image col of plane-window col 0 (window covers jbase..jbase+513)."""
    return -2 if dj > 0 else 0


def _shift_mats(spatial):
    """Banded lhsT matrices [NG, n_mats, NOUT] bf16 with spatial weights
    (divided by C_DERF) folded in. Returns (array, {(di,dj,kind): idx})."""
    def L(k, scale):
        a = np.zeros((NG, NOUT), np.float64)
        for m in range(NOUT):
            a[m + k, m] = scale
        return a

    mats, idx = [], {}
    idx["s0"] = 0
    mats.append(L(2, float(spatial[2, 2])))  # ones-stream: den += s0
    for (di, dj) in PAIRS:
        sp = float(spatial[2 + di, 2 + dj]) / C_DERF
        sm = float(spatial[2 - di, 2 - dj]) / C_DERF
        if dj == 0:
            idx[(di, dj, "den")] = len(mats)
            mats.append(L(2, sp) + L(2 - di, sm))
            idx[(di, dj, "num")] = len(mats)
            mats.append(L(2, sp) - L(2 - di, sm))
        else:
            idx[(di, dj, "A")] = len(mats)
            mats.append(L(2, sp))
            idx[(di, dj, "B")] = len(mats)
            mats.append(L(2 - di, sm))
            idx[(di, dj, "C")] = len(mats)
            mats.append(L(2 - di, -sm))
    arr = np.stack(mats, 1)  # [NG, n_mats, NOUT]
    return arr.astype(ml_dtypes.bfloat16), idx


N_MATS = 1 + 2 * 2 + 10 * 3  # 35
MAGIC = 0x7EF311C3  # fast-reciprocal seed constant


def _row_bands(h):
    bands = list(range(0, h - NOUT + 1, NOUT))
    if bands[-1] != h - NOUT:
        bands.append(h - NOUT)
    return bands


def _reflect_runs(v0, v1, h):
    """Split virtual row range [v0, v1] into runs of physical rows.
    Returns list of (p_offset, phys_start, count, step) with step +-1."""
    runs = []
    v = v0
    while v <= v1:
        if v < 0:
            e = min(-1, v1)
            runs.append((v - v0, -v, e - v + 1, -1))
            v = e + 1
        elif v >= h:
            e = v1
            runs.append((v - v0, 2 * h - 2 - v, e - v + 1, -1))
            v = e + 1
        else:
            e = min(h - 1, v1)
            runs.append((v - v0, v, e - v + 1, 1))
            v = e + 1
    return runs


def build_bilateral(nc, s0, mat_idx, h=H, w=W, b_sh=B_SH, c=C):
    """Emit the per-core program. s0 = spatial[2,2] (center weight)."""
    img_d = nc.dram_tensor("images", [b_sh, c, h, w], f32, kind="ExternalInput").ap()
    shifts_d = nc.dram_tensor(
        "shifts", [NG, N_MATS, NOUT], bf16, kind="ExternalInput"
    ).ap()
    out_d = nc.dram_tensor("out", [b_sh, c, h, w], bf16, kind="ExternalOutput").ap()

    # const APs for activation biases (0.0 for derf, s0 for Identity-add)
    for val in sorted({0.0, float(s0)}):
        key = (f32, val)
        if key not in nc.const_aps.aps:
            t = nc.alloc_sbuf_tensor(f"cbias-{val}", [128, 1], f32)
            nc.gpsimd.memset(t.ap(), val)
            nc.const_aps.aps[key] = t.ap()
    nc.all_engine_barrier()

    bands = _row_bands(h)

    with tile.TileContext(nc) as tc, ExitStack() as ctx:
        consts = ctx.enter_context(tc.tile_pool(name="consts", bufs=1))
        imgs_f = ctx.enter_context(tc.tile_pool(name="imgs_f", bufs=2))
        imgs_b = ctx.enter_context(tc.tile_pool(name="imgs_b", bufs=2))
        dpool = ctx.enter_context(tc.tile_pool(name="dpool", bufs=3))
        gpool = ctx.enter_context(tc.tile_pool(name="gpool", bufs=2))
        upool = ctx.enter_context(tc.tile_pool(name="upool", bufs=2))
        finals = ctx.enter_context(tc.tile_pool(name="finals", bufs=1))
        psums = ctx.enter_context(tc.tile_pool(name="psums", bufs=1, space="PSUM"))

        shifts = consts.tile([NG, N_MATS, NOUT], bf16)
        nc.sync.dma_start(shifts[:], shifts_d[:])

        for bi in range(b_sh):
            for r0 in bands:
                # ---- load 3 row-shifted reflect-padded f32 image copies ----
                ifs = []
                for s in range(3):
                    t = imgs_f.tile([NG, c, WB], f32, tag=f"i{s}f")
                    for (po, ps, cnt, step) in _reflect_runs(
                        r0 - 2 + s, r0 - 2 + s + NG - 1, h
                    ):
                        if step == 1:
                            src = img_d[bi, :, ps : ps + cnt, :]
                            nc.sync.dma_start(
                                t[po : po + cnt, :, 2 : 2 + w],
                                src.rearrange("c r n -> r c n"),
                            )
                        else:
                            # reflected rows: load each straight from DRAM
                            for k in range(cnt):
                                nc.sync.dma_start(
                                    t[po + k : po + k + 1, :, 2 : 2 + w],
                                    img_d[bi, :, ps - k : ps - k + 1, :].rearrange(
                                        "c r n -> r c n"
                                    ),
                                )
                    # reflect pad cols (image cols -2,-1,512,513), tiny DVE
                    for (j, jsrc) in ((0, 4), (1, 3), (2 + w, w), (3 + w, w - 1)):
                        nc.vector.tensor_copy(
                            t[:, :, j : j + 1], t[:, :, jsrc : jsrc + 1]
                        )
                    ifs.append(t)

                ibA, ibB = [], []
                for s in range(3):
                    a = imgs_b.tile([NG, c, WB], bf16, tag=f"i{s}bA")
                    nc.vector.tensor_copy(a[:], ifs[s][:])
                    ibA.append(a)
                for s in range(3):
                    b = imgs_b.tile([NG, c, WB], bf16, tag=f"i{s}bB")
                    nc.sync.dma_start(b[:, :, 0 : WB - 1], ibA[s][:, :, 1:WB])
                    ibB.append(b)

                # all subs issued up-front so the in-order DVE queue never
                # blocks a later batch's subs behind an earlier batch's umult
                dtiles = []
                for bt, batch in enumerate(BATCHES):
                    d = dpool.tile([NG, NB * c, WIN], bf16, tag="d")
                    for sl, (di, dj) in enumerate(batch):
                        jb = _jbase(dj)
                        cen = ibA[0][:, :, 2 + jb : 2 + jb + WIN]
                        if dj % 2 == 0:
                            sh = ibA[di][:, :, 2 + jb + dj : 2 + jb + dj + WIN]
                        else:
                            sh = ibB[di][:, :, 1 + jb + dj : 1 + jb + dj + WIN]
                        dsl = d[:, sl * c : (sl + 1) * c, :]
                        nc.vector.tensor_tensor(dsl, sh, cen, Alu.subtract)
                    dtiles.append(d)

                # ---- PSUM accumulators ----
                pw = psums.tile([NOUT, c, 512], f32, tag="pw")
                pa = psums.tile([NOUT, c, 512], f32, tag="pa")

                # matmuls per psum bank (channel)
                n_pw_ch = 2 * 1 + 10 * 2
                n_pa_ch = 2 * 1 + 10 * 2
                pw_cnt = [0] * c
                pa_cnt = [0] * c

                for bt, batch in enumerate(BATCHES):
                    d = dtiles[bt]
                    g = gpool.tile([NG, NB * c, WIN], bf16, tag="g")
                    u = upool.tile([NG, NB * c, WIN], bf16, tag="u")
                    # half-batch ACT + umult for finer pipelining
                    hh = NB * c // 2
                    nc.scalar.activation(
                        g[:, 0:hh, :], d[:, 0:hh, :],
                        AF.Derivative_Erf, bias=0.0, scale=SQ50,
                    )
                    nc.scalar.activation(
                        g[:, hh:, :], d[:, hh:, :],
                        AF.Derivative_Erf, bias=0.0, scale=SQ50,
                    )
                    nc.vector.tensor_tensor(
                        u[:, 0:hh, :], g[:, 0:hh, :], d[:, 0:hh, :], Alu.mult
                    )
                    nc.vector.tensor_tensor(
                        u[:, hh:, :], g[:, hh:, :], d[:, hh:, :], Alu.mult
                    )

                    # ---- PE accumulation streams ----
                    # start/stop are per PSUM zero-region (= per channel bank)
                    def mm_pw(mat, rhs):
                        k = pw_cnt[rhs_ch]
                        nc.tensor.matmul(
                            pw[:, rhs_ch, :], mat, rhs,
                            start=k == 0, stop=k == n_pw_ch - 1,
                        )
                        pw_cnt[rhs_ch] = k + 1

                    def mm_pa(mat, rhs):
                        k = pa_cnt[rhs_ch]
                        nc.tensor.matmul(
                            pa[:, rhs_ch, :], mat, rhs,
                            start=k == 0, stop=k == n_pa_ch - 1,
                        )
                        pa_cnt[rhs_ch] = k + 1

                    # pw (g-dependent) streams first: PE can start right
                    # after the ACT, overlapping the u-mults on DVE
                    for sl, (di, dj) in enumerate(batch):
                        jb = _jbase(dj)
                        od = -jb            # direct window offset in plane
                        os_ = -jb - dj      # shifted window offset
                        if dj == 0:
                            for rhs_ch in range(c):
                                mm_pw(shifts[:, mat_idx[(di, dj, "den")], :],
                                      g[:, sl * c + rhs_ch, od : od + 512])
                        else:
                            for rhs_ch in range(c):
                                mm_pw(shifts[:, mat_idx[(di, dj, "A")], :],
                                      g[:, sl * c + rhs_ch, od : od + 512])
                            for rhs_ch in range(c):
                                mm_pw(shifts[:, mat_idx[(di, dj, "B")], :],
                                      g[:, sl * c + rhs_ch, os_ : os_ + 512])
                    for sl, (di, dj) in enumerate(batch):
                        jb = _jbase(dj)
                        od = -jb
                        os_ = -jb - dj
                        if dj == 0:
                            for rhs_ch in range(c):
                                mm_pa(shifts[:, mat_idx[(di, dj, "num")], :],
                                      u[:, sl * c + rhs_ch, od : od + 512])
                        else:
                            for rhs_ch in range(c):
                                mm_pa(shifts[:, mat_idx[(di, dj, "A")], :],
                                      u[:, sl * c + rhs_ch, od : od + 512])
                            for rhs_ch in range(c):
                                mm_pa(shifts[:, mat_idx[(di, dj, "C")], :],
                                      u[:, sl * c + rhs_ch, os_ : os_ + 512])

                # ---- drain PSUM fast (ScalarE, table-swap-free) ----
                den = finals.tile([NOUT, c, 512], f32, tag="den")
                nc.scalar.add(den[:], pw[:], float(s0))      # Identity + bias
                pasb = finals.tile([NOUT, c, 512], bf16, tag="pasb")
                nc.scalar.copy(pasb[:], pa[:])

                # ---- reciprocal: bit-trick seed (DVE) + Newton (GpSimd) ----
                y0 = finals.tile([NOUT, c, 512], f32, tag="y0")
                nc.vector.tensor_scalar(
                    y0[:].bitcast(i32), den[:].bitcast(i32),
                    -1, MAGIC, Alu.mult, Alu.add,
                )
                m1 = finals.tile([NOUT, c, 512], f32, tag="m1")
                nc.gpsimd.tensor_tensor(m1[:], den[:], y0[:], Alu.mult)
                # rcpn = (m1 - 2) * y0 = -1/den (up to Newton error)
                nc.gpsimd.tensor_scalar_sub(m1[:], m1[:], 2.0)
                rcpn = finals.tile([NOUT, c, 512], f32, tag="rcpn")
                nc.gpsimd.tensor_tensor(rcpn[:], m1[:], y0[:], Alu.mult)

                # ---- finals: out = p - pa * rcpn ----
                res = finals.tile([NOUT, c, 512], f32, tag="res")
                nc.vector.tensor_tensor(res[:], pasb[:], rcpn[:], Alu.mult)
                outp = finals.tile([NOUT, c, 512], bf16, tag="outp", bufs=2)
                nc.vector.tensor_tensor(
                    outp[:], ibA[2][0:NOUT, :, 2 : 2 + w], res[:], Alu.subtract
                )
                oo = 0 if r0 == bands[0] else max(0, prev_end - r0)
                nc.sync.dma_start(
                    out_d[bi, :, r0 + oo : r0 + NOUT, :].rearrange(
                        "c r n -> r c n"
                    ),
                    outp[oo:NOUT],
                )
                prev_end = r0 + NOUT
    return nc


def make_program(spatial_kernel):
    spatial_kernel = np.asarray(spatial_kernel, dtype=np.float32)
    mats, mat_idx = _shift_mats(spatial_kernel)
    s0 = float(spatial_kernel[2, 2])
    nc = bacc.Bacc("TRN2", target_bir_lowering=False, debug=False)
    build_bilateral(nc, s0, mat_idx)
    nc.compile()
    return nc, mats


def kernel(images, spatial_kernel):
    images = np.asarray(images, dtype=np.float32)
    spatial_kernel = np.asarray(spatial_kernel, dtype=np.float32)
    nc, mats = make_program(spatial_kernel)
    in_maps = [
        {"images": images[i * B_SH : (i + 1) * B_SH], "shifts": mats}
        for i in range(N_CORES)
    ]
    res = run_bass_kernel_spmd(nc, in_maps, core_ids=list(range(N_CORES)))
    return np.concatenate(
        [res.results[i]["out"].astype(np.float32) for i in range(N_CORES)], axis=0
    )


# revision 9
# speedup vs baseline: 1.6667x; 1.6667x over previous
"""Bilateral anti-alias filter on Trainium2, 8-core data parallel.

Full inputs: images [16,3,512,512] f32, spatial_kernel [5,5] f32.
Shards the batch over 8 NeuronCores (2 images each), runs a Bass/Tile
kernel per core, gathers the full output.

Math (per pixel, K=5, sigma_i=0.1), using pair symmetry over the 12
offsets t=(di,dj) with di>=0 lexicographically positive:

  d_t = p(x+t) - p(x)
  e_t = exp(-50 d_t^2)  computed as Derivative_Erf(sqrt(50) d)*sqrt(pi)/2
  u_t = e_t * d_t
  den(x) = s0 + sum_t [s+_t e_t + s-_t shift_t(e_t)]
  pa(x)  = sum_t [s+_t u_t - s-_t shift_t(u_t)]
  out = p + pa / den

shift_t realized on the TensorEngine via banded lhsT matmuls into PSUM
(spatial weights folded into the lhsT values).  PSUM is drained fast by
ScalarE Identity/Copy (table-swap-free), the reciprocal runs on the
otherwise-idle GpSimd engine (bit-trick seed + 1 Newton step), and the
final multiply/add run on DVE.  Output is stored bf16 and upcast on the
host.
"""
import sys

sys.path.insert(0, "/opt/trn_rl_repo")

import math
import numpy as np
import ml_dtypes
from contextlib import ExitStack

import concourse.bass as bass
import concourse.tile as tile
from concourse import bacc, mybir
from concourse.bass_utils import run_bass_kernel_spmd

f32 = mybir.dt.float32
bf16 = mybir.dt.bfloat16
i32 = mybir.dt.int32
AF = mybir.ActivationFunctionType
Alu = mybir.AluOpType

N_CORES = 8
B_FULL, C, H, W = 16, 3, 512, 512
B_SH = B_FULL // N_CORES  # 2 images per core
KK = 5
PAD = KK // 2  # 2
SQ50 = float(np.sqrt(np.float32(50.0)))
C_DERF = 2.0 / math.sqrt(math.pi)  # Derivative_Erf(x) = C_DERF*exp(-x^2)
NOUT = 124  # output rows per band
NG = 128    # plane partitions (= NOUT + 4)
WB = W + 4  # 516: padded col buffer, tile col c <-> image col c-2
WIN = W + 2  # 514: per-pair plane window width

# 12 pairs (di, dj) with di >= 0, lexicographically positive
PAIRS = [
    (0, 1), (0, 2),
    (1, -2), (1, -1), (1, 0), (1, 1), (1, 2),
    (2, -2), (2, -1), (2, 0), (2, 1), (2, 2),
]
BATCHES = [PAIRS[0:4], PAIRS[4:8], PAIRS[8:12]]
NB = 4  # pairs per batch


def _jbase(dj):
    """Image col of plane-window col 0 (window covers jbase..jbase+513)."""
    return -2 if dj > 0 else 0


def _act_recip(nc, out, in_, bias):
    """rec = 1/(in_ + bias) via the ACT Reciprocal table (bass's public
    activation() refuses Reciprocal; its accuracy is ~1e-5 rel on our
    [1, 10] domain, fine for this kernel's 2e-2 budget)."""
    bias_ap = nc.const_aps.scalar_like(float(bias), in_)
    ins = [
        nc.scalar.lower_ap(in_),
        nc.scalar.lower_ap(bias_ap),
        mybir.ImmediateValue(dtype=f32, value=1.0),  # scale
        mybir.ImmediateValue(dtype=f32, value=0.0),  # alpha
    ]
    return nc.scalar.add_instruction(
        mybir.InstActivation(
            name=nc.get_next_instruction_name(),
            func=AF.Reciprocal,
            ins=ins,
            outs=[nc.scalar.lower_ap(out)],
        )
    )


def _restrict_act_tables():
    """Steer the activation-table chooser so per-band table swaps stay at
    2 (derf set <-> recip set): keep every set (indices into act_info.json
    must be preserved) but strip Derivative_Erf/Reciprocal membership from
    all other sets so they can't be chosen for them."""
    import concourse.bacc as cbacc

    if getattr(cbacc.get_activation_tables, "_bilateral_patched", False):
        return
    orig = cbacc.get_activation_tables
    keep = {
        "erf_derivative",
        "natural_log_exp_and_others",
        "reciprocal_and_small",
    }
    strip = {AF.Exp, AF.Ln, AF.Derivative_Erf, AF.Reciprocal}

    def patched(arch):
        tabs = orig(arch)
        return {
            k: (set(v) if k in keep else set(v) - strip)
            for k, v in tabs.items()
        }

    patched._bilateral_patched = True
    cbacc.get_activation_tables = patched


def _shift_mats(spatial):
    """Banded lhsT matrices [NG, n_mats, NOUT] bf16 with spatial weights
    (divided by C_DERF) folded in. Returns (array, {(di,dj,kind): idx})."""
    def L(k, scale):
        a = np.zeros((NG, NOUT), np.float64)
        for m in range(NOUT):
            a[m + k, m] = scale
        return a

    mats, idx = [], {}
    idx["s0"] = 0
    mats.append(L(2, float(spatial[2, 2])))  # ones-stream: den += s0
    for (di, dj) in PAIRS:
        sp = float(spatial[2 + di, 2 + dj]) / C_DERF
        sm = float(spatial[2 - di, 2 - dj]) / C_DERF
        if dj == 0:
            idx[(di, dj, "den")] = len(mats)
            mats.append(L(2, sp) + L(2 - di, sm))
            idx[(di, dj, "num")] = len(mats)
            mats.append(L(2, sp) - L(2 - di, sm))
        else:
            idx[(di, dj, "A")] = len(mats)
            mats.append(L(2, sp))
            idx[(di, dj, "B")] = len(mats)
            mats.append(L(2 - di, sm))
            idx[(di, dj, "C")] = len(mats)
            mats.append(L(2 - di, -sm))
    arr = np.stack(mats, 1)  # [NG, n_mats, NOUT]
    return arr.astype(ml_dtypes.bfloat16), idx


N_MATS = 1 + 2 * 2 + 10 * 3  # 35
MAGIC = 0x7EF311C3  # fast-reciprocal seed constant


def _row_bands(h):
    bands = list(range(0, h - NOUT + 1, NOUT))
    if bands[-1] != h - NOUT:
        bands.append(h - NOUT)
    return bands


def _reflect_runs(v0, v1, h):
    """Split virtual row range [v0, v1] into runs of physical rows.
    Returns list of (p_offset, phys_start, count, step) with step +-1."""
    runs = []
    v = v0
    while v <= v1:
        if v < 0:
            e = min(-1, v1)
            runs.append((v - v0, -v, e - v + 1, -1))
            v = e + 1
        elif v >= h:
            e = v1
            runs.append((v - v0, 2 * h - 2 - v, e - v + 1, -1))
            v = e + 1
        else:
            e = min(h - 1, v1)
            runs.append((v - v0, v, e - v + 1, 1))
            v = e + 1
    return runs


def build_bilateral(nc, s0, mat_idx, h=H, w=W, b_sh=B_SH, c=C):
    """Emit the per-core program. s0 = spatial[2,2] (center weight)."""
    img_d = nc.dram_tensor("images", [b_sh, c, h, w], f32, kind="ExternalInput").ap()
    shifts_d = nc.dram_tensor(
        "shifts", [NG, N_MATS, NOUT], bf16, kind="ExternalInput"
    ).ap()
    out_d = nc.dram_tensor("out", [b_sh, c, h, w], bf16, kind="ExternalOutput").ap()

    # const APs for activation biases (0.0 for derf, s0 for Identity-add)
    for val in sorted({0.0, float(s0)}):
        key = (f32, val)
        if key not in nc.const_aps.aps:
            t = nc.alloc_sbuf_tensor(f"cbias-{val}", [128, 1], f32)
            nc.gpsimd.memset(t.ap(), val)
            nc.const_aps.aps[key] = t.ap()
    nc.all_engine_barrier()

    bands = _row_bands(h)

    with tile.TileContext(nc) as tc, ExitStack() as ctx:
        consts = ctx.enter_context(tc.tile_pool(name="consts", bufs=1))
        imgs_f = ctx.enter_context(tc.tile_pool(name="imgs_f", bufs=2))
        imgs_b = ctx.enter_context(tc.tile_pool(name="imgs_b", bufs=2))
        dpool = ctx.enter_context(tc.tile_pool(name="dpool", bufs=3))
        gpool = ctx.enter_context(tc.tile_pool(name="gpool", bufs=2))
        upool = ctx.enter_context(tc.tile_pool(name="upool", bufs=2))
        finals = ctx.enter_context(tc.tile_pool(name="finals", bufs=1))
        psums = ctx.enter_context(tc.tile_pool(name="psums", bufs=1, space="PSUM"))

        shifts = consts.tile([NG, N_MATS, NOUT], bf16)
        nc.sync.dma_start(shifts[:], shifts_d[:])

        for bi in range(b_sh):
            for r0 in bands:
                # ---- load 3 row-shifted reflect-padded f32 image copies ----
                ifs = []
                for s in range(3):
                    t = imgs_f.tile([NG, c, WB], f32, tag=f"i{s}f")
                    for (po, ps, cnt, step) in _reflect_runs(
                        r0 - 2 + s, r0 - 2 + s + NG - 1, h
                    ):
                        if step == 1:
                            src = img_d[bi, :, ps : ps + cnt, :]
                            nc.sync.dma_start(
                                t[po : po + cnt, :, 2 : 2 + w],
                                src.rearrange("c r n -> r c n"),
                            )
                        else:
                            # reflected rows: load each straight from DRAM
                            for k in range(cnt):
                                nc.sync.dma_start(
                                    t[po + k : po + k + 1, :, 2 : 2 + w],
                                    img_d[bi, :, ps - k : ps - k + 1, :].rearrange(
                                        "c r n -> r c n"
                                    ),
                                )
                    # reflect pad cols (image cols -2,-1,512,513), tiny DVE
                    for (j, jsrc) in ((0, 4), (1, 3), (2 + w, w), (3 + w, w - 1)):
                        nc.vector.tensor_copy(
                            t[:, :, j : j + 1], t[:, :, jsrc : jsrc + 1]
                        )
                    ifs.append(t)

                ibA, ibB = [], []
                for s in range(3):
                    a = imgs_b.tile([NG, c, WB], bf16, tag=f"i{s}bA")
                    nc.vector.tensor_copy(a[:], ifs[s][:])
                    ibA.append(a)
                for s in range(3):
                    b = imgs_b.tile([NG, c, WB], bf16, tag=f"i{s}bB")
                    nc.sync.dma_start(b[:, :, 0 : WB - 1], ibA[s][:, :, 1:WB])
                    ibB.append(b)

                # all subs issued up-front so the in-order DVE queue never
                # blocks a later batch's subs behind an earlier batch's umult
                dtiles = []
                for bt, batch in enumerate(BATCHES):
                    d = dpool.tile([NG, NB * c, WIN], bf16, tag="d")
                    for sl, (di, dj) in enumerate(batch):
                        jb = _jbase(dj)
                        cen = ibA[0][:, :, 2 + jb : 2 + jb + WIN]
                        if dj % 2 == 0:
                            sh = ibA[di][:, :, 2 + jb + dj : 2 + jb + dj + WIN]
                        else:
                            sh = ibB[di][:, :, 1 + jb + dj : 1 + jb + dj + WIN]
                        dsl = d[:, sl * c : (sl + 1) * c, :]
                        nc.vector.tensor_tensor(dsl, sh, cen, Alu.subtract)
                    dtiles.append(d)

                # ---- PSUM accumulators ----
                pw = psums.tile([NOUT, c, 512], f32, tag="pw")
                pa = psums.tile([NOUT, c, 512], f32, tag="pa")

                # matmuls per psum bank (channel)
                n_pw_ch = 2 * 1 + 10 * 2
                n_pa_ch = 2 * 1 + 10 * 2
                pw_cnt = [0] * c
                pa_cnt = [0] * c

                for bt, batch in enumerate(BATCHES):
                    d = dtiles[bt]
                    g = gpool.tile([NG, NB * c, WIN], bf16, tag="g")
                    u = upool.tile([NG, NB * c, WIN], bf16, tag="u")
                    # half-batch ACT + umult for finer pipelining
                    hh = NB * c // 2
                    nc.scalar.activation(
                        g[:, 0:hh, :], d[:, 0:hh, :],
                        AF.Derivative_Erf, bias=0.0, scale=SQ50,
                    )
                    nc.scalar.activation(
                        g[:, hh:, :], d[:, hh:, :],
                        AF.Derivative_Erf, bias=0.0, scale=SQ50,
                    )
                    nc.vector.tensor_tensor(
                        u[:, 0:hh, :], g[:, 0:hh, :], d[:, 0:hh, :], Alu.mult
                    )
                    nc.vector.tensor_tensor(
                        u[:, hh:, :], g[:, hh:, :], d[:, hh:, :], Alu.mult
                    )

                    # ---- PE accumulation streams ----
                    # start/stop are per PSUM zero-region (= per channel bank)
                    def mm_pw(mat, rhs):
                        k = pw_cnt[rhs_ch]
                        nc.tensor.matmul(
                            pw[:, rhs_ch, :], mat, rhs,
                            start=k == 0, stop=k == n_pw_ch - 1,
                        )
                        pw_cnt[rhs_ch] = k + 1

                    def mm_pa(mat, rhs):
                        k = pa_cnt[rhs_ch]
                        nc.tensor.matmul(
                            pa[:, rhs_ch, :], mat, rhs,
                            start=k == 0, stop=k == n_pa_ch - 1,
                        )
                        pa_cnt[rhs_ch] = k + 1

                    # pw (g-dependent) streams first: PE can start right
                    # after the ACT, overlapping the u-mults on DVE
                    for sl, (di, dj) in enumerate(batch):
                        jb = _jbase(dj)
                        od = -jb            # direct window offset in plane
                        os_ = -jb - dj      # shifted window offset
                        if dj == 0:
                            for rhs_ch in range(c):
                                mm_pw(shifts[:, mat_idx[(di, dj, "den")], :],
                                      g[:, sl * c + rhs_ch, od : od + 512])
                        else:
                            for rhs_ch in range(c):
                                mm_pw(shifts[:, mat_idx[(di, dj, "A")], :],
                                      g[:, sl * c + rhs_ch, od : od + 512])
                            for rhs_ch in range(c):
                                mm_pw(shifts[:, mat_idx[(di, dj, "B")], :],
                                      g[:, sl * c + rhs_ch, os_ : os_ + 512])
                    for sl, (di, dj) in enumerate(batch):
                        jb = _jbase(dj)
                        od = -jb
                        os_ = -jb - dj
                        if dj == 0:
                            for rhs_ch in range(c):
                                mm_pa(shifts[:, mat_idx[(di, dj, "num")], :],
                                      u[:, sl * c + rhs_ch, od : od + 512])
                        else:
                            for rhs_ch in range(c):
                                mm_pa(shifts[:, mat_idx[(di, dj, "A")], :],
                                      u[:, sl * c + rhs_ch, od : od + 512])
                            for rhs_ch in range(c):
                                mm_pa(shifts[:, mat_idx[(di, dj, "C")], :],
                                      u[:, sl * c + rhs_ch, os_ : os_ + 512])

                # ---- drain PSUM (ScalarE): rec = 1/(pw+s0), pasb = bf16(pa) ----
                rec = finals.tile([NOUT, c, 512], f32, tag="rec")
                _act_recip(nc, rec[:], pw[:], float(s0))
                pasb = finals.tile([NOUT, c, 512], bf16, tag="pasb")
                nc.scalar.copy(pasb[:], pa[:])

                # ---- finals: out = p + pa * rec ----
                res = finals.tile([NOUT, c, 512], f32, tag="res")
                nc.vector.tensor_tensor(res[:], pasb[:], rec[:], Alu.mult)
                outp = finals.tile([NOUT, c, 512], bf16, tag="outp", bufs=2)
                nc.vector.tensor_tensor(
                    outp[:], ibA[2][0:NOUT, :, 2 : 2 + w], res[:], Alu.add
                )
                oo = 0 if r0 == bands[0] else max(0, prev_end - r0)
                nc.sync.dma_start(
                    out_d[bi, :, r0 + oo : r0 + NOUT, :].rearrange(
                        "c r n -> r c n"
                    ),
                    outp[oo:NOUT],
                )
                prev_end = r0 + NOUT
    return nc


def make_program(spatial_kernel):
    spatial_kernel = np.asarray(spatial_kernel, dtype=np.float32)
    mats, mat_idx = _shift_mats(spatial_kernel)
    s0 = float(spatial_kernel[2, 2])
    _restrict_act_tables()
    nc = bacc.Bacc("TRN2", target_bir_lowering=False, debug=False)
    build_bilateral(nc, s0, mat_idx)
    nc.compile()
    return nc, mats


def kernel(images, spatial_kernel):
    images = np.asarray(images, dtype=np.float32)
    spatial_kernel = np.asarray(spatial_kernel, dtype=np.float32)
    nc, mats = make_program(spatial_kernel)
    in_maps = [
        {"images": images[i * B_SH : (i + 1) * B_SH], "shifts": mats}
        for i in range(N_CORES)
    ]
    res = run_bass_kernel_spmd(nc, in_maps, core_ids=list(range(N_CORES)))
    return np.concatenate(
        [res.results[i]["out"].astype(np.float32) for i in range(N_CORES)], axis=0
    )


# revision 12
# speedup vs baseline: 1.6941x; 1.0164x over previous
"""Bilateral anti-alias filter on Trainium2, 8-core data parallel.

Full inputs: images [16,3,512,512] f32, spatial_kernel [5,5] f32.
Shards the batch over 8 NeuronCores (2 images each), runs a Bass/Tile
kernel per core, gathers the full output.

Math (per pixel, K=5, sigma_i=0.1), using pair symmetry over the 12
offsets t=(di,dj) with di>=0 lexicographically positive:

  d_t = p(x+t) - p(x)
  e_t = exp(-50 d_t^2)  computed as Derivative_Erf(sqrt(50) d)*sqrt(pi)/2
  u_t = e_t * d_t
  den(x) = s0 + sum_t [s+_t e_t + s-_t shift_t(e_t)]
  pa(x)  = sum_t [s+_t u_t - s-_t shift_t(u_t)]
  out = p + pa / den

shift_t realized on the TensorEngine via banded lhsT matmuls into PSUM
(spatial weights folded into the lhsT values).  PSUM is drained fast by
ScalarE Identity/Copy (table-swap-free), the reciprocal runs on the
otherwise-idle GpSimd engine (bit-trick seed + 1 Newton step), and the
final multiply/add run on DVE.  Output is stored bf16 and upcast on the
host.
"""
import sys

sys.path.insert(0, "/opt/trn_rl_repo")

import math
import numpy as np
import ml_dtypes
from contextlib import ExitStack

import concourse.bass as bass
import concourse.tile as tile
from concourse import bacc, mybir
from concourse.bass_utils import run_bass_kernel_spmd

f32 = mybir.dt.float32
bf16 = mybir.dt.bfloat16
i32 = mybir.dt.int32
AF = mybir.ActivationFunctionType
Alu = mybir.AluOpType

N_CORES = 8
B_FULL, C, H, W = 16, 3, 512, 512
B_SH = B_FULL // N_CORES  # 2 images per core
KK = 5
PAD = KK // 2  # 2
SQ50 = float(np.sqrt(np.float32(50.0)))
C_DERF = 2.0 / math.sqrt(math.pi)  # Derivative_Erf(x) = C_DERF*exp(-x^2)
NOUT = 124  # output rows per band
NG = 128    # plane partitions (= NOUT + 4)
WB = W + 4  # 516: padded col buffer, tile col c <-> image col c-2
WIN = W + 2  # 514: per-pair plane window width

# 12 pairs (di, dj) with di >= 0, lexicographically positive
PAIRS = [
    (0, 1), (0, 2),
    (1, -2), (1, -1), (1, 0), (1, 1), (1, 2),
    (2, -2), (2, -1), (2, 0), (2, 1), (2, 2),
]
BATCHES = [PAIRS[0:4], PAIRS[4:8], PAIRS[8:12]]
NB = 4  # pairs per batch


def _jbase(dj):
    """Image col of plane-window col 0 (window covers jbase..jbase+513)."""
    return -2 if dj > 0 else 0


def _act_recip(nc, out, in_, bias):
    """rec = 1/(in_ + bias) via the ACT Reciprocal table (bass's public
    activation() refuses Reciprocal; its accuracy is ~1e-5 rel on our
    [1, 10] domain, fine for this kernel's 2e-2 budget)."""
    bias_ap = nc.const_aps.scalar_like(float(bias), in_)
    ins = [
        nc.scalar.lower_ap(in_),
        nc.scalar.lower_ap(bias_ap),
        mybir.ImmediateValue(dtype=f32, value=1.0),  # scale
        mybir.ImmediateValue(dtype=f32, value=0.0),  # alpha
    ]
    return nc.scalar.add_instruction(
        mybir.InstActivation(
            name=nc.get_next_instruction_name(),
            func=AF.Reciprocal,
            ins=ins,
            outs=[nc.scalar.lower_ap(out)],
        )
    )


def _restrict_act_tables():
    """Steer the activation-table chooser so per-band table swaps stay at
    2 (derf set <-> recip set): keep every set (indices into act_info.json
    must be preserved) but strip Derivative_Erf/Reciprocal membership from
    all other sets so they can't be chosen for them."""
    import concourse.bacc as cbacc

    if getattr(cbacc.get_activation_tables, "_bilateral_patched", False):
        return
    orig = cbacc.get_activation_tables
    keep = {
        "erf_derivative",
        "natural_log_exp_and_others",
        "reciprocal_and_small",
    }
    strip = {AF.Exp, AF.Ln, AF.Derivative_Erf, AF.Reciprocal}

    def patched(arch):
        tabs = orig(arch)
        return {
            k: (set(v) if k in keep else set(v) - strip)
            for k, v in tabs.items()
        }

    patched._bilateral_patched = True
    cbacc.get_activation_tables = patched


def _shift_mats(spatial):
    """Banded lhsT matrices [NG, n_mats, NOUT] bf16 with spatial weights
    (divided by C_DERF) folded in. Returns (array, {(di,dj,kind): idx})."""
    def L(k, scale):
        a = np.zeros((NG, NOUT), np.float64)
        for m in range(NOUT):
            a[m + k, m] = scale
        return a

    mats, idx = [], {}
    idx["s0"] = 0
    mats.append(L(2, float(spatial[2, 2])))  # ones-stream: den += s0
    for (di, dj) in PAIRS:
        sp = float(spatial[2 + di, 2 + dj]) / C_DERF
        sm = float(spatial[2 - di, 2 - dj]) / C_DERF
        if dj == 0:
            idx[(di, dj, "den")] = len(mats)
            mats.append(L(2, sp) + L(2 - di, sm))
            idx[(di, dj, "num")] = len(mats)
            mats.append(L(2, sp) - L(2 - di, sm))
        else:
            idx[(di, dj, "A")] = len(mats)
            mats.append(L(2, sp))
            idx[(di, dj, "B")] = len(mats)
            mats.append(L(2 - di, sm))
            idx[(di, dj, "C")] = len(mats)
            mats.append(L(2 - di, -sm))
    arr = np.stack(mats, 1)  # [NG, n_mats, NOUT]
    return arr.astype(ml_dtypes.bfloat16), idx


N_MATS = 1 + 2 * 2 + 10 * 3  # 35
MAGIC = 0x7EF311C3  # fast-reciprocal seed constant


def _row_bands(h):
    bands = list(range(0, h - NOUT + 1, NOUT))
    if bands[-1] != h - NOUT:
        bands.append(h - NOUT)
    return bands


def _reflect_runs(v0, v1, h):
    """Split virtual row range [v0, v1] into runs of physical rows.
    Returns list of (p_offset, phys_start, count, step) with step +-1."""
    runs = []
    v = v0
    while v <= v1:
        if v < 0:
            e = min(-1, v1)
            runs.append((v - v0, -v, e - v + 1, -1))
            v = e + 1
        elif v >= h:
            e = v1
            runs.append((v - v0, 2 * h - 2 - v, e - v + 1, -1))
            v = e + 1
        else:
            e = min(h - 1, v1)
            runs.append((v - v0, v, e - v + 1, 1))
            v = e + 1
    return runs


def build_bilateral(nc, s0, mat_idx, h=H, w=W, b_sh=B_SH, c=C):
    """Emit the per-core program. s0 = spatial[2,2] (center weight)."""
    img_d = nc.dram_tensor("images", [b_sh, c, h, w], f32, kind="ExternalInput").ap()
    shifts_d = nc.dram_tensor(
        "shifts", [NG, N_MATS, NOUT], bf16, kind="ExternalInput"
    ).ap()
    out_d = nc.dram_tensor("out", [b_sh, c, h, w], bf16, kind="ExternalOutput").ap()

    # const APs for activation biases (0.0 for derf, s0 for Identity-add)
    for val in sorted({0.0, float(s0)}):
        key = (f32, val)
        if key not in nc.const_aps.aps:
            t = nc.alloc_sbuf_tensor(f"cbias-{val}", [128, 1], f32)
            nc.gpsimd.memset(t.ap(), val)
            nc.const_aps.aps[key] = t.ap()
    nc.all_engine_barrier()

    bands = _row_bands(h)

    with tile.TileContext(nc) as tc, ExitStack() as ctx:
        consts = ctx.enter_context(tc.tile_pool(name="consts", bufs=1))
        imgs_f = ctx.enter_context(tc.tile_pool(name="imgs_f", bufs=2))
        imgs_b = ctx.enter_context(tc.tile_pool(name="imgs_b", bufs=2))
        dpool = ctx.enter_context(tc.tile_pool(name="dpool", bufs=3))
        gpool = ctx.enter_context(tc.tile_pool(name="gpool", bufs=2))
        upool = ctx.enter_context(tc.tile_pool(name="upool", bufs=2))
        finals = ctx.enter_context(tc.tile_pool(name="finals", bufs=1))
        psums = ctx.enter_context(tc.tile_pool(name="psums", bufs=1, space="PSUM"))

        shifts = consts.tile([NG, N_MATS, NOUT], bf16)
        nc.sync.dma_start(shifts[:], shifts_d[:])

        for bi in range(b_sh):
            for r0 in bands:
                # ---- load 3 row-shifted reflect-padded f32 image copies ----
                ifs = []
                for s in range(3):
                    t = imgs_f.tile([NG, c, WB], f32, tag=f"i{s}f")
                    for (po, ps, cnt, step) in _reflect_runs(
                        r0 - 2 + s, r0 - 2 + s + NG - 1, h
                    ):
                        if step == 1:
                            # per-channel DMAs: each instruction rides one
                            # SDMA engine (~27 GiB/s), so splitting puts 3
                            # engines on the tile and cuts latency 3x
                            for ch in range(c):
                                nc.sync.dma_start(
                                    t[po : po + cnt, ch, 2 : 2 + w],
                                    img_d[bi, ch, ps : ps + cnt, :],
                                )
                        else:
                            # reflected rows: load each straight from DRAM
                            for k in range(cnt):
                                nc.sync.dma_start(
                                    t[po + k : po + k + 1, :, 2 : 2 + w],
                                    img_d[bi, :, ps - k : ps - k + 1, :].rearrange(
                                        "c r n -> r c n"
                                    ),
                                )
                    # reflect pad cols (image cols -2,-1,512,513), tiny DVE
                    for (j, jsrc) in ((0, 4), (1, 3), (2 + w, w), (3 + w, w - 1)):
                        nc.vector.tensor_copy(
                            t[:, :, j : j + 1], t[:, :, jsrc : jsrc + 1]
                        )
                    ifs.append(t)

                ibA, ibB = [], []
                for s in range(3):
                    a = imgs_b.tile([NG, c, WB], bf16, tag=f"i{s}bA")
                    nc.vector.tensor_copy(a[:], ifs[s][:])
                    ibA.append(a)
                for s in range(3):
                    b = imgs_b.tile([NG, c, WB], bf16, tag=f"i{s}bB")
                    nc.sync.dma_start(b[:, :, 0 : WB - 1], ibA[s][:, :, 1:WB])
                    ibB.append(b)

                # all subs issued up-front so the in-order DVE queue never
                # blocks a later batch's subs behind an earlier batch's umult
                dtiles = []
                for bt, batch in enumerate(BATCHES):
                    d = dpool.tile([NG, NB * c, WIN], bf16, tag="d")
                    for sl, (di, dj) in enumerate(batch):
                        jb = _jbase(dj)
                        cen = ibA[0][:, :, 2 + jb : 2 + jb + WIN]
                        if dj % 2 == 0:
                            sh = ibA[di][:, :, 2 + jb + dj : 2 + jb + dj + WIN]
                        else:
                            sh = ibB[di][:, :, 1 + jb + dj : 1 + jb + dj + WIN]
                        dsl = d[:, sl * c : (sl + 1) * c, :]
                        nc.vector.tensor_tensor(dsl, sh, cen, Alu.subtract)
                    dtiles.append(d)

                # ---- PSUM accumulators ----
                pw = psums.tile([NOUT, c, 512], f32, tag="pw")
                pa = psums.tile([NOUT, c, 512], f32, tag="pa")

                # matmuls per psum bank (channel)
                n_pw_ch = 2 * 1 + 10 * 2
                n_pa_ch = 2 * 1 + 10 * 2
                pw_cnt = [0] * c
                pa_cnt = [0] * c

                for bt, batch in enumerate(BATCHES):
                    d = dtiles[bt]
                    g = gpool.tile([NG, NB * c, WIN], bf16, tag="g")
                    u = upool.tile([NG, NB * c, WIN], bf16, tag="u")
                    # half-batch ACT + umult for finer pipelining
                    hh = NB * c // 2
                    nc.scalar.activation(
                        g[:, 0:hh, :], d[:, 0:hh, :],
                        AF.Derivative_Erf, bias=0.0, scale=SQ50,
                    )
                    nc.scalar.activation(
                        g[:, hh:, :], d[:, hh:, :],
                        AF.Derivative_Erf, bias=0.0, scale=SQ50,
                    )
                    nc.vector.tensor_tensor(
                        u[:, 0:hh, :], g[:, 0:hh, :], d[:, 0:hh, :], Alu.mult
                    )
                    nc.vector.tensor_tensor(
                        u[:, hh:, :], g[:, hh:, :], d[:, hh:, :], Alu.mult
                    )

                    # ---- PE accumulation streams ----
                    # start/stop are per PSUM zero-region (= per channel bank)
                    def mm_pw(mat, rhs):
                        k = pw_cnt[rhs_ch]
                        nc.tensor.matmul(
                            pw[:, rhs_ch, :], mat, rhs,
                            start=k == 0, stop=k == n_pw_ch - 1,
                        )
                        pw_cnt[rhs_ch] = k + 1

                    def mm_pa(mat, rhs):
                        k = pa_cnt[rhs_ch]
                        nc.tensor.matmul(
                            pa[:, rhs_ch, :], mat, rhs,
                            start=k == 0, stop=k == n_pa_ch - 1,
                        )
                        pa_cnt[rhs_ch] = k + 1

                    # pw (g-dependent) streams first: PE can start right
                    # after the ACT, overlapping the u-mults on DVE
                    for sl, (di, dj) in enumerate(batch):
                        jb = _jbase(dj)
                        od = -jb            # direct window offset in plane
                        os_ = -jb - dj      # shifted window offset
                        if dj == 0:
                            for rhs_ch in range(c):
                                mm_pw(shifts[:, mat_idx[(di, dj, "den")], :],
                                      g[:, sl * c + rhs_ch, od : od + 512])
                        else:
                            for rhs_ch in range(c):
                                mm_pw(shifts[:, mat_idx[(di, dj, "A")], :],
                                      g[:, sl * c + rhs_ch, od : od + 512])
                            for rhs_ch in range(c):
                                mm_pw(shifts[:, mat_idx[(di, dj, "B")], :],
                                      g[:, sl * c + rhs_ch, os_ : os_ + 512])
                    for sl, (di, dj) in enumerate(batch):
                        jb = _jbase(dj)
                        od = -jb
                        os_ = -jb - dj
                        if dj == 0:
                            for rhs_ch in range(c):
                                mm_pa(shifts[:, mat_idx[(di, dj, "num")], :],
                                      u[:, sl * c + rhs_ch, od : od + 512])
                        else:
                            for rhs_ch in range(c):
                                mm_pa(shifts[:, mat_idx[(di, dj, "A")], :],
                                      u[:, sl * c + rhs_ch, od : od + 512])
                            for rhs_ch in range(c):
                                mm_pa(shifts[:, mat_idx[(di, dj, "C")], :],
                                      u[:, sl * c + rhs_ch, os_ : os_ + 512])

                # ---- drain PSUM (ScalarE): rec = 1/(pw+s0), pasb = bf16(pa) ----
                rec = finals.tile([NOUT, c, 512], bf16, tag="rec")
                _act_recip(nc, rec[:], pw[:], float(s0))
                pasb = finals.tile([NOUT, c, 512], bf16, tag="pasb")
                nc.scalar.copy(pasb[:], pa[:])

                # ---- finals: out = p + pa * rec ----
                res = finals.tile([NOUT, c, 512], bf16, tag="res")
                nc.vector.tensor_tensor(res[:], pasb[:], rec[:], Alu.mult)
                outp = finals.tile([NOUT, c, 512], bf16, tag="outp", bufs=2)
                nc.vector.tensor_tensor(
                    outp[:], ibA[2][0:NOUT, :, 2 : 2 + w], res[:], Alu.add
                )
                oo = 0 if r0 == bands[0] else max(0, prev_end - r0)
                nc.sync.dma_start(
                    out_d[bi, :, r0 + oo : r0 + NOUT, :].rearrange(
                        "c r n -> r c n"
                    ),
                    outp[oo:NOUT],
                )
                prev_end = r0 + NOUT
    return nc


def make_program(spatial_kernel):
    spatial_kernel = np.asarray(spatial_kernel, dtype=np.float32)
    mats, mat_idx = _shift_mats(spatial_kernel)
    s0 = float(spatial_kernel[2, 2])
    _restrict_act_tables()
    nc = bacc.Bacc("TRN2", target_bir_lowering=False, debug=False)
    build_bilateral(nc, s0, mat_idx)
    nc.compile()
    return nc, mats


def kernel(images, spatial_kernel):
    images = np.asarray(images, dtype=np.float32)
    spatial_kernel = np.asarray(spatial_kernel, dtype=np.float32)
    nc, mats = make_program(spatial_kernel)
    in_maps = [
        {"images": images[i * B_SH : (i + 1) * B_SH], "shifts": mats}
        for i in range(N_CORES)
    ]
    res = run_bass_kernel_spmd(nc, in_maps, core_ids=list(range(N_CORES)))
    return np.concatenate(
        [res.results[i]["out"].astype(np.float32) for i in range(N_CORES)], axis=0
    )
